# revision 81
# baseline (speedup 1.0000x reference)
"""Trainium2 Bass kernel for causal multi-head attention.

Problem: x[1,4096,1024] -> MHA(16 heads, head_dim 64, causal) -> out[1,4096,1024]
  q,k,v = x @ W_{q,k,v}; scores = q k^T / 8 (causal); out = softmax(scores) v @ W_o + b_o

Sharding: tensor-parallel over heads, 2 heads (128 feature dims) per core.
Each core produces a full-width partial output ctx_c @ W_o[slice_c] which the
host sums over the 8 cores (row-parallel out-projection).

Per-core dataflow (single software-pipelined loop over 512-row chunks, the
next chunk's projections and the previous chunk's out-projection paced evenly
through the current chunk's attention steps):
  - ~5us of dummy identity matmuls at program start warm the PE HAM clock
    gate to 8/8 (2.4 GHz) while the first x chunk DMAs in (chunk-major DRAM
    layout, one DMA instruction per chunk -- issue costs ~0.6us each on the
    serial sync queue).
  - Q/K projections as fp8e4m3 DoubleRow matmuls (x and W quantized on host;
    scales folded so fp8 stays clear of subnormals and exp() compensates with
    an exact power-of-two argument scale). V projection stays bf16 -- V-path
    noise does not average away relative to the 1/sqrt(k)-shrinking context
    (fp8 anywhere on the P/V path measures ~3e-2 max-rel: over the gate).
  - Q/K land with both heads stacked on partitions (h0 0:64, h1 64:128) so
    the two heads' score matmuls (K=64 contraction each) run CONCURRENTLY as
    PE row-tiles at tile_position (0,0)/(64,0), halving score PE time.
  - Scores S^T = K Q^T in bf16, column-restricted on the causal diagonal.
  - exp() alternates per-tile between ACT (exact, 8/12) and DVE (Schraudolph
    bit-trick bf16-bits via int16 bitcast, 4/12); the last 8 tiles of every
    chunk (and the whole last chunk) alternate strictly 1:1, diag tiles
    included, so each chunk-end LAG-deep PV flush drains at the full
    two-engine stream rate. (Schraudolph share is error-free at this
    tolerance: fp8 q/k dominates the 1.03e-2 max-rel error, gate is 2e-2.)
  - Causal masking of diagonal tiles on GPSIMD (affine_select).
  - P V^T and the softmax row-sums in one bf16 matmul per (tile, head) via a
    ones-augmented V stationary, column-restricted on the diagonal; PV
    emission trails the exp stream by LAG=8 tiles so the PE's in-order queue
    never stalls on the previous chunk's pv drain.
  - Per-q softmax normalization per head (reciprocal of the replicated sums
    rows, then fused into the ctx eviction) so the next chunk's head-0 PV
    unblocks after half the DVE chain.
  - Out-projection at full 128-deep contraction in bf16; PSUM evicted by
    ACT/DVE copies; the last chunk's out DMAs issue from the scalar/gpsimd
    queues to avoid serializing on sync at the kernel tail.

kernel(**inputs) takes the FULL unsharded inputs and returns the FULL output.
"""

import sys

import numpy as np

for _p in ("/opt/trn_rl_repo", "/root/.axon_site/_ro/trn_rl_repo"):
    if _p not in sys.path:
        try:
            import concourse  # noqa: F401

            break
        except ImportError:
            sys.path.insert(0, _p)

N_CORES = 8
SEQ = 4096
D = 1024
DC = 128  # per-core slice of the head dim (2 heads x 64)
HD = 64

# host-side scale folding
QW_SCALE = 8.0       # W_q * (1/sqrt(hd)) * 64
KW_SCALE = 64.0
VW_SCALE = 64.0      # folded back out through W_o
EXP_SCALE = 1.0 / 4096.0
SCH_A = (128.0 / np.log(2.0)) * EXP_SCALE  # schraudolph multiplier (bf16 bits)
SCH_B = 16248.75                           # schraudolph bias, zero-mean tuned

# engine-balance knobs
DVE_EXP_NUM = 4    # of every DVE_EXP_DEN exp tiles, this many go to the DVE
DVE_EXP_DEN = 12


def build_bass(n=SEQ, d=D):
    import concourse.bacc as bacc
    import concourse.mybir as mybir
    import concourse.tile as tile
    from concourse.masks import make_identity

    fp32 = mybir.dt.float32
    bf16 = mybir.dt.bfloat16
    fp8 = mybir.dt.float8e4
    i16 = mybir.dt.int16
    Exp = mybir.ActivationFunctionType.Exp
    Copy = mybir.ActivationFunctionType.Copy
    DR = mybir.MatmulPerfMode.DoubleRow
    mult = mybir.AluOpType.mult
    add = mybir.AluOpType.add

    assert n % 512 == 0 and d % 128 == 0
    NCH = n // 512   # 512-col seq chunks
    DIT = d // 128   # input-dim 128-tiles (8)

    nc = bacc.Bacc("TRN2", target_bir_lowering=False)

    # chunk-major DRAM layout: per-chunk DMA reads are per-partition contiguous
    xb_d = nc.dram_tensor("xb", (NCH, 128, DIT, 512), bf16, kind="ExternalInput")
    xq_d = nc.dram_tensor("xq", (NCH, 128, DIT, 512), fp8, kind="ExternalInput")
    wq_d = nc.dram_tensor("wq", (128, DIT, DC), fp8, kind="ExternalInput")
    wk_d = nc.dram_tensor("wk", (128, DIT, DC), fp8, kind="ExternalInput")
    wv_d = nc.dram_tensor("wv", (128, DIT, DC), bf16, kind="ExternalInput")
    wo_d = nc.dram_tensor("wo", (DC, d), bf16, kind="ExternalInput")
    out_d = nc.dram_tensor("out", (n, d), bf16, kind="ExternalOutput")

    with tile.TileContext(nc) as tc:
        with (
            tc.tile_pool(name="const", bufs=1) as const_pool,
            tc.tile_pool(name="weights", bufs=1) as w_pool,
            tc.tile_pool(name="big", bufs=1) as big_pool,
            tc.tile_pool(name="vt", bufs=4) as vt_pool,
            tc.tile_pool(name="pm", bufs=15) as pm_pool,
            tc.tile_pool(name="recip", bufs=4) as r_pool,
            tc.tile_pool(name="ctxr", bufs=6) as cx_pool,
            tc.tile_pool(name="outsb", bufs=7) as out_pool,
            tc.tile_pool(name="psA", bufs=1, space="PSUM") as psA,
            tc.tile_pool(name="psB", bufs=1, space="PSUM") as psB,
        ):
            # ---- constants ----
            ident = const_pool.tile([128, 128], bf16)
            make_identity(nc, ident[:])

            # ---- PE warmup: ~5us of dummy matmuls during the initial x DMA
            # wait, so the HAM clock-gate reaches 8/8 (2.4 GHz) before the
            # first projection and the PE never sits cold behind the DMA ----
            warm = psA.tile([128, 2, 512], fp32, tag="s", bufs=3, name="warm")
            for _ in range(40):
                nc.tensor.matmul(
                    warm[:, 0, 0:128], ident[:], ident[:], start=True, stop=True
                )

            # ---- weights + x (fp8 for q/k DoubleRow, bf16 for v) ----
            wq8 = w_pool.tile([128, DIT, DC], fp8)
            wk8 = w_pool.tile([128, DIT, DC], fp8)
            wvb = w_pool.tile([128, DIT, DC], bf16)
            nc.sync.dma_start(wq8[:], wq_d[:])
            nc.sync.dma_start(wk8[:], wk_d[:])
            wo_sb = w_pool.tile([DC, d], bf16)
            xb = big_pool.tile([128, DIT, n], bf16)
            x8 = big_pool.tile([128, DIT, n], fp8)
            # one DMA per chunk (DMA-issue instructions cost ~0.6us each on the
            # serial sync queue); each chunk's fp8 (q/k path) lands before its
            # bf16 (v path). wv/wo queue AFTER chunk 0 -- they are not needed
            # until part_v/the first out-proj, and ahead of x8[0] they delay
            # the first projection by ~2us.
            nc.sync.dma_start(x8[:, :, 0:512], xq_d[0])
            nc.sync.dma_start(xb[:, :, 0:512], xb_d[0])
            nc.sync.dma_start(wvb[:], wv_d[:])
            nc.sync.dma_start(wo_sb[:], wo_d[:])
            for c in range(1, NCH):
                cs = slice(c * 512, (c + 1) * 512)
                nc.sync.dma_start(x8[:, :, cs], xq_d[c])
                nc.sync.dma_start(xb[:, :, cs], xb_d[c])

            # ---- persistent per-chunk activations ----
            # heads stacked on partitions (h0 on 0:64, h1 on 64:128) so the two
            # heads' score matmuls run CONCURRENTLY as PE row-tiles (K=64 each,
            # tile_position (0,0)/(64,0) auto-derived from base partitions).
            # qt/kt MUST be separate tiles: sharing one tile creates false
            # cross-plane dependencies in the tracker (+35us measured).
            qt_c = [big_pool.tile([128, 512], bf16, name=f"qt{c}") for c in range(NCH)]
            kt_c = [big_pool.tile([128, 512], bf16, name=f"kt{c}") for c in range(NCH)]
            # V stationary per head: 128 wide -- ones BLOCK at cols 0:64 (the
            # PV matmul then replicates the softmax sums across partitions
            # 0:64 for free), V dims at cols 64:128 (ctx lands on partitions
            # 64:128; DVE 64-partition PSUM reads must start at 0 or 64).
            va = big_pool.tile([128, 4 * NCH, 256], bf16, name="va")
            nc.gpsimd.memset(va[:, :, 0:HD], 1.0)
            nc.gpsimd.memset(va[:, :, 128 : 128 + HD], 1.0)

            exp_ctr = [0]

            def emit_proj(c):
                """Projections + V transpose for chunk c (interleavable parts)."""
                parts = []
                xs = slice(c * 512, c * 512 + 512)

                def part_q():
                    qp = psA.tile([128, 2, 512], fp32, tag="s", bufs=3, name="qp")
                    for p in range(DIT // 2):
                        nc.tensor.matmul(
                            qp[:, 0, :], wq8[:, 2 * p : 2 * p + 2, :],
                            x8[:, 2 * p : 2 * p + 2, xs],
                            start=(p == 0), stop=(p == DIT // 2 - 1),
                            perf_mode=DR,
                        )
                    nc.scalar.activation(qt_c[c][:], qp[:, 0, :], Copy)

                def part_k():
                    kp = psA.tile([128, 2, 512], fp32, tag="s", bufs=3, name="kp")
                    for p in range(DIT // 2):
                        nc.tensor.matmul(
                            kp[:, 0, :], wk8[:, 2 * p : 2 * p + 2, :],
                            x8[:, 2 * p : 2 * p + 2, xs],
                            start=(p == 0), stop=(p == DIT // 2 - 1),
                            perf_mode=DR,
                        )
                    nc.vector.tensor_copy(kt_c[c][:], kp[:, 0, :])

                vstate = []

                def part_v1():
                    vp = psA.tile([128, 2, 512], fp32, tag="s", bufs=3, name="vp")
                    vstate.append(vp)
                    for t in range(DIT // 2):
                        nc.tensor.matmul(
                            vp[:, 0, :], wvb[:, t, :], xb[:, t, xs],
                            start=(t == 0), stop=False,
                        )

                def part_v2():
                    vp = vstate[0]
                    for t in range(DIT // 2, DIT):
                        nc.tensor.matmul(
                            vp[:, 0, :], wvb[:, t, :], xb[:, t, xs],
                            start=False, stop=(t == DIT - 1),
                        )
                    vt_t = vt_pool.tile([DC, 512], bf16, tag="vt", bufs=4)
                    nc.vector.tensor_copy(vt_t[:], vp[:, 0, :])
                    vstate.append(vt_t)

                def make_part_t(j0):
                    def part_t():
                        vt_t = vstate[1]
                        for j in (j0, j0 + 1):
                            tp = psA.tile(
                                [128, 128], bf16, tag="s", bufs=3,
                                padded_shape=[128, 2048], name="tp",
                            )
                            nc.tensor.transpose(
                                tp[:], vt_t[:, j * 128 : (j + 1) * 128], ident[:]
                            )
                            # segmented copy: cols 0:64 -> 32:96, 64:128 -> 128:192
                            nc.vector.tensor_copy(
                                va[:, 4 * c + j, :].rearrange(
                                    "p (h w) -> p h w", h=2
                                )[:, :, HD:DC],
                                tp[:].rearrange("p (h w) -> p h w", h=2),
                            )
                    return part_t

                return [part_q, part_k, part_v1, part_v2,
                        make_part_t(0), make_part_t(2)]

            def emit_attn(qc, inter_early, inter_late):
                """Causal attention + out-projection for q chunk qc."""
                nkt = 4 * (qc + 1)
                pv = psB.tile([128, 2, 512], fp32, tag="pv", bufs=1, name="pv")
                inter = list(inter_early) + list(inter_late)
                n_inter_tot = [len(inter)]
                n_inter_done = [0]
                LAG = 9  # PV trails the exp stream so the prior chunk's
                         # normalization chain never stalls the PE queue
                pend_pv = []

                def emit_pv(kt, pm):
                    dd = kt - 4 * qc
                    off = 128 * dd if dd > 0 else 0
                    for h in range(2):
                        nc.tensor.matmul(
                            pv[:, h, off:],
                            va[:, kt, 128 * h : 128 * h + 128],
                            pm[:, h, off:],
                            start=(kt == 0), stop=(kt == nkt - 1),
                        )

                for kt in range(nkt):
                    kc = slice((kt % 4) * 128, (kt % 4) * 128 + 128)
                    ktc = kt // 4
                    dd = kt - 4 * qc
                    off = 128 * dd if dd > 0 else 0
                    sm = psA.tile([128, 2, 512], fp32, tag="s", bufs=3, name="sm")
                    for h in range(2):
                        hs = slice(HD * h, HD * h + HD)
                        nc.tensor.matmul(
                            sm[:, h, off:],
                            kt_c[ktc][hs, kc], qt_c[qc][hs, off:],
                            start=True, stop=True,
                        )
                    pm = pm_pool.tile([128, 2, 512], bf16, tag="pm", bufs=15)
                    use_dve = (exp_ctr[0] * DVE_EXP_NUM) // DVE_EXP_DEN != (
                        (exp_ctr[0] + 1) * DVE_EXP_NUM
                    ) // DVE_EXP_DEN
                    exp_ctr[0] += 1
                    if qc == NCH - 1 or kt >= nkt - 8:
                        # near every chunk end (and the whole last chunk):
                        # strict per-tile ACT/DVE alternation (diag tiles
                        # included) so the LAG-deep PV flush drains at the
                        # full two-engine exp stream rate
                        if kt % 2 == 1:
                            nc.vector.tensor_scalar(
                                pm[:, :, off:].bitcast(i16), sm[:, :, off:],
                                SCH_A, SCH_B, mult, add,
                            )
                        else:
                            nc.scalar.activation(
                                pm[:, :, off:], sm[:, :, off:], Exp,
                                scale=EXP_SCALE,
                            )
                    elif use_dve and dd < 0:
                        nc.vector.tensor_scalar(
                            pm[:].bitcast(i16), sm[:],
                            SCH_A, SCH_B, mult, add,
                        )
                    else:
                        nc.scalar.activation(
                            pm[:, :, off:], sm[:, :, off:], Exp, scale=EXP_SCALE
                        )
                    if dd >= 0:
                        # causal triangle within the diagonal 128-col band;
                        # columns beyond the band are fully valid
                        nc.gpsimd.affine_select(
                            out=pm[:, :, off : off + 128], in_=pm[:, :, off : off + 128],
                            compare_op=mybir.AluOpType.is_ge,
                            fill=0.0, base=0,
                            pattern=[[0, 2], [1, 128]],
                            channel_multiplier=-1,
                        )
                    pend_pv.append((kt, pm))
                    if len(pend_pv) > LAG:
                        emit_pv(*pend_pv.pop(0))
                    # pace the interleaved proj/out-proj parts evenly across
                    # the whole chunk so the late steps (and the PV flush)
                    # still have PE filler work
                    while inter and n_inter_done[0] < (
                        (kt + 1) * n_inter_tot[0]
                    ) // nkt:
                        inter.pop(0)()
                        n_inter_done[0] += 1
                for args in pend_pv:
                    emit_pv(*args)
                for part in inter:
                    part()
                inter = []

                # softmax normalization fused into the ctx eviction: pv rows
                # 0:64 hold 64 replicas of the sums row (ones-block stationary),
                # so one approx-reciprocal yields the broadcast directly.
                # Per-head order so the next chunk's head-0 PV (same PSUM bank)
                # unblocks after only half the chain.
                rbc = r_pool.tile([HD, 2, 512], fp32, tag="rbc", bufs=4)
                ctxr = cx_pool.tile([DC, 512], bf16, tag="cx", bufs=6)
                nc.vector.reciprocal_approx_fast(rbc[:, 0, :], pv[0:HD, 0, :])
                nc.vector.tensor_mul(ctxr[0:HD, :], pv[HD:DC, 0, :], rbc[:, 0, :])
                nc.vector.reciprocal_approx_fast(rbc[:, 1, :], pv[0:HD, 1, :])
                nc.vector.tensor_mul(ctxr[HD:DC, :], pv[HD:DC, 1, :], rbc[:, 1, :])

                # out-projection parts, deferred into the next chunk's late
                # interleave slots so scores fill the normalization-chain gap;
                # the two output halves evict on ACT and DVE in parallel
                def op_part(j):
                    def part():
                        gsl = slice(qc * 512 + j * 128, qc * 512 + j * 128 + 128)
                        op = psA.tile([128, 2, 512], fp32, tag="s", bufs=3, name="op")
                        for h2 in range(2):
                            nc.tensor.matmul(
                                op[:, h2, :],
                                ctxr[:, j * 128 : (j + 1) * 128],
                                wo_sb[:, h2 * 512 : (h2 + 1) * 512],
                                start=True, stop=True,
                            )
                        o_sb = out_pool.tile([128, 2, 512], bf16, tag="o", bufs=7)
                        nc.scalar.activation(o_sb[:, 0, :], op[:, 0, :], Copy)
                        nc.vector.tensor_copy(o_sb[:, 1, :], op[:, 1, :])
                        if qc == NCH - 1:
                            # kernel tail: issue each half's DMA from the
                            # engine that produced it (idle by now) instead of
                            # serializing ~0.6us-each issues on sync
                            nc.scalar.dma_start(out_d[gsl, 0:512], o_sb[:, 0, :])
                            nc.gpsimd.dma_start(
                                out_d[gsl, 512:1024], o_sb[:, 1, :]
                            )
                        else:
                            for h2 in range(2):
                                nc.sync.dma_start(
                                    out_d[gsl, h2 * 512 : (h2 + 1) * 512],
                                    o_sb[:, h2, :],
                                )
                    return part

                return [op_part(j) for j in range(4)]

            # ---- pipelined main loop ----
            for p in emit_proj(0):
                p()
            pending = []
            for qc in range(NCH):
                proj_parts = emit_proj(qc + 1) if qc + 1 < NCH else []
                pending = emit_attn(qc, proj_parts, pending)
            for p in pending:
                p()

    nc.compile()
    return nc


_NC_CACHE = {}


def _get_nc(n=SEQ):
    if n not in _NC_CACHE:
        _NC_CACHE[n] = build_bass(n)
    return _NC_CACHE[n]


def make_in_maps(x, W_q, W_k, W_v, W_o):
    import ml_dtypes

    fp8 = ml_dtypes.float8_e4m3
    bf16 = ml_dtypes.bfloat16
    n = x.shape[-2]
    xT = np.asarray(x, dtype=np.float32).reshape(n, D).T  # [d, n]
    xr = np.ascontiguousarray(xT.reshape(D // 128, 128, n).transpose(1, 0, 2))
    # chunk-major [NCH, 128, DIT, 512]
    xcm = np.ascontiguousarray(
        xr.reshape(128, D // 128, n // 512, 512).transpose(2, 0, 1, 3)
    )
    xb = xcm.astype(bf16)
    xq = xcm.astype(fp8)

    def wprep(w, s, sl, dt):
        w = np.asarray(w, np.float32)[:, sl] * s  # [d, DC]
        return np.ascontiguousarray(
            w.reshape(D // 128, 128, DC).transpose(1, 0, 2)
        ).astype(dt)

    in_maps = []
    for c in range(N_CORES):
        sl = slice(c * DC, (c + 1) * DC)
        in_maps.append(
            {
                "xb": xb,
                "xq": xq,
                "wq": wprep(W_q, QW_SCALE, sl, fp8),
                "wk": wprep(W_k, KW_SCALE, sl, fp8),
                "wv": wprep(W_v, VW_SCALE, sl, bf16),
                "wo": np.ascontiguousarray(
                    np.asarray(W_o, np.float32)[sl, :] / VW_SCALE
                ).astype(bf16),
            }
        )
    return in_maps


def kernel(x, W_q, W_k, W_v, W_o, b_o):
    from concourse import bass_utils

    x = np.asarray(x)
    b, n, _ = x.shape
    assert b == 1 and n == SEQ

    nc = _get_nc(n)
    in_maps = make_in_maps(x, W_q, W_k, W_v, W_o)
    res = bass_utils.run_bass_kernel_spmd(nc, in_maps, list(range(N_CORES)))
    acc = np.zeros((n, D), dtype=np.float64)
    for r in res.results:
        acc += r["out"].astype(np.float64)
    acc += np.asarray(b_o, np.float64)[None, :]
    return acc.astype(np.float32).reshape(1, n, D)



# revision 82
# speedup vs baseline: 1.0028x; 1.0028x over previous
"""Trainium2 Bass kernel for causal multi-head attention.

Problem: x[1,4096,1024] -> MHA(16 heads, head_dim 64, causal) -> out[1,4096,1024]
  q,k,v = x @ W_{q,k,v}; scores = q k^T / 8 (causal); out = softmax(scores) v @ W_o + b_o

Sharding: tensor-parallel over heads, 2 heads (128 feature dims) per core.
Each core produces a full-width partial output ctx_c @ W_o[slice_c] which the
host sums over the 8 cores (row-parallel out-projection).

Per-core dataflow (single software-pipelined loop over 512-row chunks, the
next chunk's projections and the previous chunk's out-projection paced evenly
through the current chunk's attention steps):
  - ~5us of dummy identity matmuls at program start warm the PE HAM clock
    gate to 8/8 (2.4 GHz) while the first x chunk DMAs in (chunk-major DRAM
    layout, one DMA instruction per chunk -- issue costs ~0.6us each on the
    serial sync queue).
  - Q/K projections as fp8e4m3 DoubleRow matmuls (x and W quantized on host;
    scales folded so fp8 stays clear of subnormals and exp() compensates with
    an exact power-of-two argument scale). V projection stays bf16 -- V-path
    noise does not average away relative to the 1/sqrt(k)-shrinking context
    (fp8 anywhere on the P/V path measures ~3e-2 max-rel: over the gate).
  - Q/K land with both heads stacked on partitions (h0 0:64, h1 64:128) so
    the two heads' score matmuls (K=64 contraction each) run CONCURRENTLY as
    PE row-tiles at tile_position (0,0)/(64,0), halving score PE time.
  - Scores S^T = K Q^T in bf16, column-restricted on the causal diagonal.
  - exp() alternates per-tile between ACT (exact, 8/12) and DVE (Schraudolph
    bit-trick bf16-bits via int16 bitcast, 4/12); the last 8 tiles of every
    chunk (and the whole last chunk) alternate strictly 1:1, diag tiles
    included, so each chunk-end LAG-deep PV flush drains at the full
    two-engine stream rate. (Schraudolph share is error-free at this
    tolerance: fp8 q/k dominates the 1.03e-2 max-rel error, gate is 2e-2.)
  - Causal masking of diagonal tiles on GPSIMD (affine_select).
  - P V^T and the softmax row-sums in one bf16 matmul per (tile, head) via a
    ones-augmented V stationary, column-restricted on the diagonal; PV
    emission trails the exp stream by LAG=8 tiles so the PE's in-order queue
    never stalls on the previous chunk's pv drain.
  - Per-q softmax normalization per head (reciprocal of the replicated sums
    rows, then fused into the ctx eviction) so the next chunk's head-0 PV
    unblocks after half the DVE chain.
  - Out-projection at full 128-deep contraction in bf16; PSUM evicted by
    ACT/DVE copies; the last chunk's out DMAs issue from the scalar/gpsimd
    queues to avoid serializing on sync at the kernel tail.

kernel(**inputs) takes the FULL unsharded inputs and returns the FULL output.
"""

import sys

import numpy as np

for _p in ("/opt/trn_rl_repo", "/root/.axon_site/_ro/trn_rl_repo"):
    if _p not in sys.path:
        try:
            import concourse  # noqa: F401

            break
        except ImportError:
            sys.path.insert(0, _p)

N_CORES = 8
SEQ = 4096
D = 1024
DC = 128  # per-core slice of the head dim (2 heads x 64)
HD = 64

# host-side scale folding
QW_SCALE = 8.0       # W_q * (1/sqrt(hd)) * 64
KW_SCALE = 64.0
VW_SCALE = 64.0      # folded back out through W_o
EXP_SCALE = 1.0 / 4096.0
SCH_A = (128.0 / np.log(2.0)) * EXP_SCALE  # schraudolph multiplier (bf16 bits)
SCH_B = 16248.75                           # schraudolph bias, zero-mean tuned

# engine-balance knobs
DVE_EXP_NUM = 4    # of every DVE_EXP_DEN exp tiles, this many go to the DVE
DVE_EXP_DEN = 12


def build_bass(n=SEQ, d=D):
    import concourse.bacc as bacc
    import concourse.mybir as mybir
    import concourse.tile as tile
    from concourse.masks import make_identity

    fp32 = mybir.dt.float32
    bf16 = mybir.dt.bfloat16
    fp8 = mybir.dt.float8e4
    i16 = mybir.dt.int16
    Exp = mybir.ActivationFunctionType.Exp
    Copy = mybir.ActivationFunctionType.Copy
    DR = mybir.MatmulPerfMode.DoubleRow
    mult = mybir.AluOpType.mult
    add = mybir.AluOpType.add

    assert n % 512 == 0 and d % 128 == 0
    NCH = n // 512   # 512-col seq chunks
    DIT = d // 128   # input-dim 128-tiles (8)

    nc = bacc.Bacc("TRN2", target_bir_lowering=False)

    # chunk-major DRAM layout: per-chunk DMA reads are per-partition contiguous
    xb_d = nc.dram_tensor("xb", (NCH, 128, DIT, 512), bf16, kind="ExternalInput")
    xq_d = nc.dram_tensor("xq", (NCH, 128, DIT, 512), fp8, kind="ExternalInput")
    wq_d = nc.dram_tensor("wq", (128, DIT, DC), fp8, kind="ExternalInput")
    wk_d = nc.dram_tensor("wk", (128, DIT, DC), fp8, kind="ExternalInput")
    wv_d = nc.dram_tensor("wv", (128, DIT, DC), bf16, kind="ExternalInput")
    wo_d = nc.dram_tensor("wo", (DC, d), bf16, kind="ExternalInput")
    out_d = nc.dram_tensor("out", (n, d), bf16, kind="ExternalOutput")

    with tile.TileContext(nc) as tc:
        with (
            tc.tile_pool(name="const", bufs=1) as const_pool,
            tc.tile_pool(name="weights", bufs=1) as w_pool,
            tc.tile_pool(name="big", bufs=1) as big_pool,
            tc.tile_pool(name="vt", bufs=3) as vt_pool,
            tc.tile_pool(name="pm", bufs=15) as pm_pool,
            tc.tile_pool(name="recip", bufs=4) as r_pool,
            tc.tile_pool(name="ctxr", bufs=6) as cx_pool,
            tc.tile_pool(name="outsb", bufs=7) as out_pool,
            tc.tile_pool(name="psA", bufs=1, space="PSUM") as psA,
            tc.tile_pool(name="psB", bufs=1, space="PSUM") as psB,
        ):
            # ---- constants ----
            ident = const_pool.tile([128, 128], bf16)
            make_identity(nc, ident[:])

            # ---- PE warmup: ~5us of dummy matmuls during the initial x DMA
            # wait, so the HAM clock-gate reaches 8/8 (2.4 GHz) before the
            # first projection and the PE never sits cold behind the DMA ----
            warm = psA.tile([128, 2, 512], fp32, tag="s", bufs=3, name="warm")
            for _ in range(40):
                nc.tensor.matmul(
                    warm[:, 0, 0:128], ident[:], ident[:], start=True, stop=True
                )

            # ---- weights + x (fp8 for q/k DoubleRow, bf16 for v) ----
            wq8 = w_pool.tile([128, DIT, DC], fp8)
            wk8 = w_pool.tile([128, DIT, DC], fp8)
            wvb = w_pool.tile([128, DIT, DC], bf16)
            nc.sync.dma_start(wq8[:], wq_d[:])
            nc.sync.dma_start(wk8[:], wk_d[:])
            wo_sb = w_pool.tile([DC, d], bf16)
            xb = big_pool.tile([128, DIT, n], bf16)
            x8 = big_pool.tile([128, DIT, n], fp8)
            # one DMA per chunk (DMA-issue instructions cost ~0.6us each on the
            # serial sync queue); each chunk's fp8 (q/k path) lands before its
            # bf16 (v path). wv/wo queue AFTER chunk 0 -- they are not needed
            # until part_v/the first out-proj, and ahead of x8[0] they delay
            # the first projection by ~2us.
            nc.sync.dma_start(x8[:, :, 0:512], xq_d[0])
            nc.sync.dma_start(xb[:, :, 0:512], xb_d[0])
            nc.sync.dma_start(wvb[:], wv_d[:])
            nc.sync.dma_start(wo_sb[:], wo_d[:])
            for c in range(1, NCH):
                cs = slice(c * 512, (c + 1) * 512)
                nc.sync.dma_start(x8[:, :, cs], xq_d[c])
                nc.sync.dma_start(xb[:, :, cs], xb_d[c])

            # ---- persistent per-chunk activations ----
            # heads stacked on partitions (h0 on 0:64, h1 on 64:128) so the two
            # heads' score matmuls run CONCURRENTLY as PE row-tiles (K=64 each,
            # tile_position (0,0)/(64,0) auto-derived from base partitions).
            # qt/kt MUST be separate tiles: sharing one tile creates false
            # cross-plane dependencies in the tracker (+35us measured).
            qt_c = [big_pool.tile([128, 512], bf16, name=f"qt{c}") for c in range(NCH)]
            kt_c = [big_pool.tile([128, 512], bf16, name=f"kt{c}") for c in range(NCH)]
            # V stationary per head: 128 wide -- ones BLOCK at cols 0:64 (the
            # PV matmul then replicates the softmax sums across partitions
            # 0:64 for free), V dims at cols 64:128 (ctx lands on partitions
            # 64:128; DVE 64-partition PSUM reads must start at 0 or 64).
            va = big_pool.tile([128, 4 * NCH, 256], bf16, name="va")
            nc.gpsimd.memset(va[:, :, 0:HD], 1.0)
            nc.gpsimd.memset(va[:, :, 128 : 128 + HD], 1.0)

            exp_ctr = [0]

            def emit_proj(c):
                """Projections + V transpose for chunk c (interleavable parts)."""
                parts = []
                xs = slice(c * 512, c * 512 + 512)

                def part_q():
                    qp = psA.tile([128, 2, 512], fp32, tag="s", bufs=3, name="qp")
                    for p in range(DIT // 2):
                        nc.tensor.matmul(
                            qp[:, 0, :], wq8[:, 2 * p : 2 * p + 2, :],
                            x8[:, 2 * p : 2 * p + 2, xs],
                            start=(p == 0), stop=(p == DIT // 2 - 1),
                            perf_mode=DR,
                        )
                    nc.scalar.activation(qt_c[c][:], qp[:, 0, :], Copy)

                def part_k():
                    kp = psA.tile([128, 2, 512], fp32, tag="s", bufs=3, name="kp")
                    for p in range(DIT // 2):
                        nc.tensor.matmul(
                            kp[:, 0, :], wk8[:, 2 * p : 2 * p + 2, :],
                            x8[:, 2 * p : 2 * p + 2, xs],
                            start=(p == 0), stop=(p == DIT // 2 - 1),
                            perf_mode=DR,
                        )
                    nc.vector.tensor_copy(kt_c[c][:], kp[:, 0, :])

                vstate = []

                def part_v1():
                    vp = psA.tile([128, 2, 512], fp32, tag="s", bufs=3, name="vp")
                    vstate.append(vp)
                    for t in range(DIT // 2):
                        nc.tensor.matmul(
                            vp[:, 0, :], wvb[:, t, :], xb[:, t, xs],
                            start=(t == 0), stop=False,
                        )

                def part_v2():
                    vp = vstate[0]
                    for t in range(DIT // 2, DIT):
                        nc.tensor.matmul(
                            vp[:, 0, :], wvb[:, t, :], xb[:, t, xs],
                            start=False, stop=(t == DIT - 1),
                        )
                    vt_t = vt_pool.tile([DC, 512], bf16, tag="vt", bufs=3)
                    nc.vector.tensor_copy(vt_t[:], vp[:, 0, :])
                    vstate.append(vt_t)

                def make_part_t(j0):
                    def part_t():
                        vt_t = vstate[1]
                        for j in (j0, j0 + 1):
                            tp = psA.tile(
                                [128, 128], bf16, tag="s", bufs=3,
                                padded_shape=[128, 2048], name="tp",
                            )
                            nc.tensor.transpose(
                                tp[:], vt_t[:, j * 128 : (j + 1) * 128], ident[:]
                            )
                            # segmented copy: cols 0:64 -> 32:96, 64:128 -> 128:192
                            nc.vector.tensor_copy(
                                va[:, 4 * c + j, :].rearrange(
                                    "p (h w) -> p h w", h=2
                                )[:, :, HD:DC],
                                tp[:].rearrange("p (h w) -> p h w", h=2),
                            )
                    return part_t

                return [part_q, part_k, part_v1, part_v2,
                        make_part_t(0), make_part_t(2)]

            def emit_attn(qc, inter_early, inter_late):
                """Causal attention + out-projection for q chunk qc."""
                nkt = 4 * (qc + 1)
                pv = psB.tile([128, 2, 512], fp32, tag="pv", bufs=1, name="pv")
                inter = list(inter_early) + list(inter_late)
                n_inter_tot = [len(inter)]
                n_inter_done = [0]
                LAG = 9  # PV trails the exp stream so the prior chunk's
                         # normalization chain never stalls the PE queue
                pend_pv = []

                def emit_pv(kt, pm):
                    dd = kt - 4 * qc
                    off = 128 * dd if dd > 0 else 0
                    for h in range(2):
                        nc.tensor.matmul(
                            pv[:, h, off:],
                            va[:, kt, 128 * h : 128 * h + 128],
                            pm[:, h, off:],
                            start=(kt == 0), stop=(kt == nkt - 1),
                        )

                for kt in range(nkt):
                    kc = slice((kt % 4) * 128, (kt % 4) * 128 + 128)
                    ktc = kt // 4
                    dd = kt - 4 * qc
                    off = 128 * dd if dd > 0 else 0
                    sm = psA.tile([128, 2, 512], fp32, tag="s", bufs=3, name="sm")
                    for h in range(2):
                        hs = slice(HD * h, HD * h + HD)
                        nc.tensor.matmul(
                            sm[:, h, off:],
                            kt_c[ktc][hs, kc], qt_c[qc][hs, off:],
                            start=True, stop=True,
                        )
                    pm = pm_pool.tile([128, 2, 512], bf16, tag="pm", bufs=15)
                    use_dve = (exp_ctr[0] * DVE_EXP_NUM) // DVE_EXP_DEN != (
                        (exp_ctr[0] + 1) * DVE_EXP_NUM
                    ) // DVE_EXP_DEN
                    exp_ctr[0] += 1
                    if qc == NCH - 1 or kt >= nkt - 8:
                        # near every chunk end (and the whole last chunk):
                        # strict per-tile ACT/DVE alternation (diag tiles
                        # included) so the LAG-deep PV flush drains at the
                        # full two-engine exp stream rate
                        if kt % 2 == 1:
                            nc.vector.tensor_scalar(
                                pm[:, :, off:].bitcast(i16), sm[:, :, off:],
                                SCH_A, SCH_B, mult, add,
                            )
                        else:
                            nc.scalar.activation(
                                pm[:, :, off:], sm[:, :, off:], Exp,
                                scale=EXP_SCALE,
                            )
                    elif use_dve and dd < 0:
                        nc.vector.tensor_scalar(
                            pm[:].bitcast(i16), sm[:],
                            SCH_A, SCH_B, mult, add,
                        )
                    else:
                        nc.scalar.activation(
                            pm[:, :, off:], sm[:, :, off:], Exp, scale=EXP_SCALE
                        )
                    if dd >= 0:
                        # causal triangle within the diagonal 128-col band;
                        # columns beyond the band are fully valid
                        nc.gpsimd.affine_select(
                            out=pm[:, :, off : off + 128], in_=pm[:, :, off : off + 128],
                            compare_op=mybir.AluOpType.is_ge,
                            fill=0.0, base=0,
                            pattern=[[0, 2], [1, 128]],
                            channel_multiplier=-1,
                        )
                    pend_pv.append((kt, pm))
                    if len(pend_pv) > LAG:
                        emit_pv(*pend_pv.pop(0))
                    # pace the interleaved proj/out-proj parts evenly across
                    # the whole chunk so the late steps (and the PV flush)
                    # still have PE filler work
                    while inter and n_inter_done[0] < (
                        (kt + 1) * n_inter_tot[0]
                    ) // nkt:
                        inter.pop(0)()
                        n_inter_done[0] += 1
                for args in pend_pv:
                    emit_pv(*args)
                for part in inter:
                    part()
                inter = []

                # softmax normalization fused into the ctx eviction: pv rows
                # 0:64 hold 64 replicas of the sums row (ones-block stationary),
                # so one approx-reciprocal yields the broadcast directly.
                # Per-head order so the next chunk's head-0 PV (same PSUM bank)
                # unblocks after only half the chain.
                rbc = r_pool.tile([HD, 2, 512], fp32, tag="rbc", bufs=4)
                ctxr = cx_pool.tile([DC, 512], bf16, tag="cx", bufs=6)
                nc.vector.reciprocal_approx_fast(rbc[:, 0, :], pv[0:HD, 0, :])
                nc.vector.tensor_mul(ctxr[0:HD, :], pv[HD:DC, 0, :], rbc[:, 0, :])
                nc.vector.reciprocal_approx_fast(rbc[:, 1, :], pv[0:HD, 1, :])
                nc.vector.tensor_mul(ctxr[HD:DC, :], pv[HD:DC, 1, :], rbc[:, 1, :])

                # out-projection parts, deferred into the next chunk's late
                # interleave slots so scores fill the normalization-chain gap;
                # the two output halves evict on ACT and DVE in parallel
                def op_part(j):
                    def part():
                        gsl = slice(qc * 512 + j * 128, qc * 512 + j * 128 + 128)
                        op = psA.tile([128, 2, 512], fp32, tag="s", bufs=3, name="op")
                        for h2 in range(2):
                            nc.tensor.matmul(
                                op[:, h2, :],
                                ctxr[:, j * 128 : (j + 1) * 128],
                                wo_sb[:, h2 * 512 : (h2 + 1) * 512],
                                start=True, stop=True,
                            )
                        o_sb = out_pool.tile([128, 2, 512], bf16, tag="o", bufs=7)
                        nc.scalar.activation(o_sb[:, 0, :], op[:, 0, :], Copy)
                        nc.vector.tensor_copy(o_sb[:, 1, :], op[:, 1, :])
                        if qc == NCH - 1:
                            # kernel tail: issue each half's DMA from the
                            # engine that produced it (idle by now) instead of
                            # serializing ~0.6us-each issues on sync
                            nc.scalar.dma_start(out_d[gsl, 0:512], o_sb[:, 0, :])
                            nc.gpsimd.dma_start(
                                out_d[gsl, 512:1024], o_sb[:, 1, :]
                            )
                        else:
                            for h2 in range(2):
                                nc.sync.dma_start(
                                    out_d[gsl, h2 * 512 : (h2 + 1) * 512],
                                    o_sb[:, h2, :],
                                )
                    return part

                return [op_part(j) for j in range(4)]

            # ---- pipelined main loop ----
            for p in emit_proj(0):
                p()
            pending = []
            for qc in range(NCH):
                proj_parts = emit_proj(qc + 1) if qc + 1 < NCH else []
                pending = emit_attn(qc, proj_parts, pending)
            for p in pending:
                p()

    nc.compile()
    return nc


_NC_CACHE = {}


def _get_nc(n=SEQ):
    if n not in _NC_CACHE:
        _NC_CACHE[n] = build_bass(n)
    return _NC_CACHE[n]


def make_in_maps(x, W_q, W_k, W_v, W_o):
    import ml_dtypes

    fp8 = ml_dtypes.float8_e4m3
    bf16 = ml_dtypes.bfloat16
    n = x.shape[-2]
    xT = np.asarray(x, dtype=np.float32).reshape(n, D).T  # [d, n]
    xr = np.ascontiguousarray(xT.reshape(D // 128, 128, n).transpose(1, 0, 2))
    # chunk-major [NCH, 128, DIT, 512]
    xcm = np.ascontiguousarray(
        xr.reshape(128, D // 128, n // 512, 512).transpose(2, 0, 1, 3)
    )
    xb = xcm.astype(bf16)
    xq = xcm.astype(fp8)

    def wprep(w, s, sl, dt):
        w = np.asarray(w, np.float32)[:, sl] * s  # [d, DC]
        return np.ascontiguousarray(
            w.reshape(D // 128, 128, DC).transpose(1, 0, 2)
        ).astype(dt)

    in_maps = []
    for c in range(N_CORES):
        sl = slice(c * DC, (c + 1) * DC)
        in_maps.append(
            {
                "xb": xb,
                "xq": xq,
                "wq": wprep(W_q, QW_SCALE, sl, fp8),
                "wk": wprep(W_k, KW_SCALE, sl, fp8),
                "wv": wprep(W_v, VW_SCALE, sl, bf16),
                "wo": np.ascontiguousarray(
                    np.asarray(W_o, np.float32)[sl, :] / VW_SCALE
                ).astype(bf16),
            }
        )
    return in_maps


def kernel(x, W_q, W_k, W_v, W_o, b_o):
    from concourse import bass_utils

    x = np.asarray(x)
    b, n, _ = x.shape
    assert b == 1 and n == SEQ

    nc = _get_nc(n)
    in_maps = make_in_maps(x, W_q, W_k, W_v, W_o)
    res = bass_utils.run_bass_kernel_spmd(nc, in_maps, list(range(N_CORES)))
    acc = np.zeros((n, D), dtype=np.float64)
    for r in res.results:
        acc += r["out"].astype(np.float64)
    acc += np.asarray(b_o, np.float64)[None, :]
    return acc.astype(np.float32).reshape(1, n, D)

